# revision 11
# baseline (speedup 1.0000x reference)
"""CSGNet (gnn_message_passing) Trainium2 kernel, v3.

Sharding (per hint): data-parallel over graphs, 32 graphs per core.

Phase 1 (scatter-free aggregation): the host sorts edges by destination
node and pads each node's edge list to K slots (K = next pow2 >= max
degree), so GraphConv aggregation becomes a dense per-node row sum. The
device streams fp16 source-value and weight arrays, multiplies (DVE 2x
mode), and sums the K slots with a pairwise add tree (tensor_tensor gets
the 2x fp16 mode; tensor_reduce does not).

Phase 2: GraphConv combine in fp16 on DVE; relu + LayerNorm stats fused
on the scalar engine (accum_out); LayerNorm folded into conv1 (channel-
wise ln_g/ln_b checked on host). conv1/conv2 run on the PE as block-
diagonal matmuls over PE-transposed per-graph tiles (6-node packing);
the LN affine + relu epilogue is fused into scalar-engine activations
reading PSUM. FC stack on PE with fp16 fc_w1. Eval-BatchNorms folded on
host.
"""

import numpy as np

import concourse.bass as bass
import concourse.mybir as mybir
from concourse.tile import TileContext
from concourse.vector_clock import ScopedClock
from concourse.bass_utils import run_bass_kernel_spmd

F32 = mybir.dt.float32
F16 = mybir.dt.float16
OP = mybir.AluOpType
AX = mybir.AxisListType
AF = mybir.ActivationFunctionType

B, N, M = 256, 2207, 16
C1, C2 = 12, 4
H1, H2 = 256, 64
EPS = 1e-5
BN_SCALE = 1.0 / np.sqrt(1.0 + 1e-5)
NCORES = 8

NF = 18                      # node j of graph g at (j % 128, NF*g + j // 128)
NPAD = NF * 128              # 2304 padded nodes per graph
CW = 48                      # node-columns per phase-1 chunk (576 / 12)
TRACE = False                # capture NTFF profile (test harness only)
LAST = {}                    # test harness: last run artifacts


# ---------------------------------------------------------------------------
# workaround: this walrus build rejects >2 sem waits on one TPB_CTRL
# instruction; spread the TileContext tail-drain waits over 1-wait nops.
def _patched_drain_and_barrier(self, tick_clock, wait_clock):
    probe = self.nc.sync.nop(nofuse=True)
    wait_clock.add_sem_waits(probe.ins, ScopedClock({None: tick_clock.global_clock}))
    si = probe.ins.sync_info
    waits = list(si.on_wait) if si is not None and si.on_wait else []
    if len(waits) > 1:
        si.on_wait.clear()
        si.on_wait.append(waits[0])
        for w in waits[1:]:
            n2 = self.nc.sync.nop(nofuse=True)
            n2.ins.sync_info = mybir.SyncInfo(on_wait=[w], on_update=[])
    self.nc.sync.drain()
    self.nc.all_engine_barrier()
    popped = self.nc._tile_sem_poison_stack.pop()
    assert popped is self._sem_poison
    self.nc.clear_and_free_semaphores(list(self.sems.allocated().values()))
    self.nc.all_engine_barrier()


TileContext._drain_and_barrier = _patched_drain_and_barrier


def _split_excess_waits(nc, limit=1):
    """Walrus caps sem waits per instruction; move extras to same-engine
    nops placed immediately before the offending instruction."""
    n = 0
    for fn in nc.m.functions:
        for bb in fn.blocks:
            insts = bb.instructions
            out = []
            changed = False
            for inst in insts:
                si = inst.sync_info
                if si is not None and si.on_wait and len(si.on_wait) > limit:
                    waits = list(si.on_wait)
                    extra, keep = waits[:-limit], waits[-limit:]
                    for i in range(0, len(extra), limit):
                        n += 1
                        out.append(mybir.InstNoOp(
                            name=f"ZZwait-{n}", engine=inst.engine,
                            sync_info=mybir.SyncInfo(
                                on_wait=extra[i:i + limit], on_update=[])))
                    inst.sync_info = mybir.SyncInfo(
                        on_wait=keep, on_update=list(si.on_update or []))
                    changed = True
                out.append(inst)
            if changed:
                bb.instructions = out
# ---------------------------------------------------------------------------


def _build_program(gpc, K, pad_s, pad_q):
    """SPMD Tile program. gpc graphs/core, K slots/node (power of 2)."""
    GF = gpc * NF               # 576 node-columns per core
    nch = GF // CW
    assert GF % CW == 0 and (K & (K - 1)) == 0

    nc = bass.Bass()
    dp = lambda n, s, d=F32: nc.declare_dram_parameter(n, s, d, isOutput=False)

    vx = dp("vx", [128, GF * K], F16)
    vw = dp("vw", [128, GF * K], F16)
    x128 = dp("x128", [128, GF], F16)
    wrel = dp("wrel", [128, M])
    wroot = dp("wroot", [128, M])
    brel = dp("brel", [128, M])
    bd1 = dp("bd1", [96, 6 * C1], F16)    # blockdiag W1' (LN-gamma folded)
    bd2 = dp("bd2", [6 * C1, 6 * C2], F16)  # blockdiag BN1-folded gc2_w
    cw1po = dp("cw1po", [128, 1])         # row sums of W1' at part 12w+o
    b1ppo = dp("b1ppo", [128, 1])         # b1' at part 12w+o
    b2po = dp("b2po", [128, 1])           # folded conv2 bias at part 4w+c
    fw1 = dp("fw1", [128, (C2 * NF) * H1], F16)
    fb1 = dp("fb1", [1, H1])
    fw2 = dp("fw2", [128, 2 * H2])
    fb2 = dp("fb2", [1, H2])
    fw3 = dp("fw3", [64, 1])
    ident = dp("ident", [128, 128])
    ident16 = dp("ident16", [128, 128], F16)
    ones = dp("ones", [128, 1])
    ones_row = dp("ones_row", [1, 128])
    fb3 = dp("fb3", [128, 1])
    out_p = nc.declare_dram_parameter("out", [gpc, 1], F32, isOutput=True)

    with TileContext(nc) as tc:
        with (
            tc.tile_pool(name="const", bufs=1) as cpool,
            tc.tile_pool(name="main", bufs=1) as mp,
        ):
            # big weight prefetch first so it overlaps phase 1
            fw1_sb = cpool.tile([128, (C2 * NF) * H1], F16)
            nc.sync.dma_start(out=fw1_sb[:], in_=fw1[:])

            def ld(t, shape, dt=F32):
                s = cpool.tile(list(shape), dt, tag=t.name)
                nc.sync.dma_start(out=s[:], in_=t[:])
                return s

            ident_sb = ld(ident, [128, 128])
            id16_sb = ld(ident16, [128, 128], F16)
            ones_sb = ld(ones, [128, 1])
            onesr_sb = ld(ones_row, [1, 128])
            x_sb = ld(x128, [128, GF], F16)
            wrel_sb = ld(wrel, [128, M])
            wroot_sb = ld(wroot, [128, M])
            brel_sb = ld(brel, [128, M])
            bd1_sb = ld(bd1, [96, 6 * C1], F16)
            bd2_sb = ld(bd2, [6 * C1, 6 * C2], F16)
            cw1po_sb = ld(cw1po, [128, 1])
            b1ppo_sb = ld(b1ppo, [128, 1])
            b2po_sb = ld(b2po, [128, 1])
            fb1_sb = ld(fb1, [1, H1])
            fw2_sb = ld(fw2, [128, 2 * H2])
            fb2_sb = ld(fb2, [1, H2])
            fw3_sb = ld(fw3, [64, 1])
            fb3_sb = ld(fb3, [128, 1])

            agg = mp.tile([128, GF], F32, tag="agg")

            # -------- Phase 1: fp16 multiply + pairwise-tree slot sums -----
            with (
                tc.tile_pool(name="edges", bufs=3) as epool,
                tc.tile_pool(name="prod", bufs=2) as ppool,
            ):
                for ch in range(nch):
                    c0 = ch * CW
                    vx_t = epool.tile([128, CW * K], F16, tag="vx")
                    vw_t = epool.tile([128, CW * K], F16, tag="vw")
                    nc.sync.dma_start(
                        out=vx_t[:], in_=vx[:, c0 * K:(c0 + CW) * K])
                    nc.sync.dma_start(
                        out=vw_t[:], in_=vw[:, c0 * K:(c0 + CW) * K])
                    prod = ppool.tile([128, CW * K], F16, tag="prod")
                    nc.vector.tensor_mul(out=prod[:], in0=vx_t[:], in1=vw_t[:])
                    p3 = prod[:].rearrange("q (c k) -> q c k", k=K)
                    with nc.allow_low_precision(reason="fp16 slot-sum tree"):
                        k = K
                        while k > 2:
                            h = k // 2
                            nc.vector.tensor_tensor(
                                out=p3[:, :, 0:h], in0=p3[:, :, 0:h],
                                in1=p3[:, :, h:k], op=OP.add)
                            k = h
                    nc.vector.tensor_tensor(
                        out=agg[:, c0:c0 + CW], in0=p3[:, :, 0],
                        in1=p3[:, :, 1], op=OP.add)

            # -------- Phase 2: combine, LN stats, PE convs, FC stack -------
            agg16 = mp.tile([128, GF], F16, tag="agg16")
            nc.vector.tensor_copy(out=agg16[:], in_=agg[:])

            # z[m, n] = agg[n]*wrel[m] + x[n]*wroot[m] + brel[m]  (pre-relu)
            h5 = mp.tile([128, M * GF], F16, tag="h5")      # [q, m, gf]
            h5v = h5[:].rearrange("q (m gf) -> q m gf", m=M)
            tmp = mp.tile([128, GF], F16, tag="tmp")
            for m in range(M):
                nc.vector.tensor_scalar(
                    out=tmp[:], in0=x_sb[:],
                    scalar1=wroot_sb[:, m:m + 1], scalar2=brel_sb[:, m:m + 1],
                    op0=OP.mult, op1=OP.add)
                nc.vector.scalar_tensor_tensor(
                    out=h5v[:, m], in0=agg16[:],
                    scalar=wrel_sb[:, m:m + 1], in1=tmp[:],
                    op0=OP.mult, op1=OP.add)

            # relu + per-graph LN stats on the scalar engine (accum_out)
            hr5 = mp.tile([128, M * GF], F16, tag="hr5")
            hr5v = hr5[:].rearrange("q (gf m) -> q gf m", m=M)
            ssum = mp.tile([128, 2 * gpc], F32, tag="ssum")
            with tc.tile_pool(name="sq", bufs=2) as sqpool:
                for g in range(gpc):
                    sl = slice(g * NF, (g + 1) * NF)
                    nc.scalar.activation(
                        out=hr5v[:, sl].rearrange("q f m -> q m f"),
                        in_=h5v[:, :, sl], func=AF.Relu,
                        accum_out=ssum[:, g:g + 1])
                    sq = sqpool.tile([128, M * NF], F16, tag="sq")
                    nc.scalar.activation(
                        out=sq[:].rearrange("q (f m) -> q f m", m=M),
                        in_=hr5v[:, sl], func=AF.Square,
                        accum_out=ssum[:, gpc + g:gpc + g + 1])

            # mual[128, 64] = alpha | alpha*mu (broadcast over partitions)
            ps1_cm = tc.tile_pool(name="ps1", bufs=1, space="PSUM")
            ps1 = ps1_cm.__enter__()
            pstat = ps1.tile([1, 2 * gpc], F32, tag="psA")
            nc.tensor.matmul(out=pstat[:], lhsT=ones_sb[:], rhs=ssum[:],
                             start=True, stop=True)
            inv = 1.0 / (N * M)
            mu1 = mp.tile([1, gpc], F32, tag="mu1")
            nc.vector.tensor_scalar(
                out=mu1[:], in0=pstat[:, 0:gpc],
                scalar1=-pad_s, scalar2=inv, op0=OP.add, op1=OP.mult)
            e2 = mp.tile([1, gpc], F32, tag="e2")
            nc.vector.tensor_scalar(
                out=e2[:], in0=pstat[:, gpc:2 * gpc],
                scalar1=-pad_q, scalar2=inv, op0=OP.add, op1=OP.mult)
            musq = mp.tile([1, gpc], F32, tag="musq")
            nc.vector.tensor_mul(out=musq[:], in0=mu1[:], in1=mu1[:])
            nc.vector.tensor_sub(out=e2[:], in0=e2[:], in1=musq[:])
            nc.vector.tensor_scalar(
                out=e2[:], in0=e2[:], scalar1=EPS, scalar2=None, op0=OP.add)
            sd1 = mp.tile([1, gpc], F32, tag="sd1")
            nc.scalar.sqrt(out=sd1[:], in_=e2[:])
            mual1 = mp.tile([1, 2 * gpc], F32, tag="mual1")
            nc.vector.reciprocal(out=mual1[:, 0:gpc], in_=sd1[:])
            nc.vector.tensor_mul(out=mual1[:, gpc:2 * gpc],
                                 in0=mual1[:, 0:gpc], in1=mu1[:])
            mualp = ps1.tile([128, 2 * gpc], F32, tag="psA2")
            nc.tensor.matmul(out=mualp[:], lhsT=onesr_sb[:], rhs=mual1[:],
                             start=True, stop=True)
            mual = mp.tile([128, 2 * gpc], F32, tag="mual")
            nc.vector.tensor_copy(out=mual[:], in_=mualp[:])
            ps1_cm.__exit__(None, None, None)

            # conv1 epilogue bias D[12w+o, g] = b1p[o] - alpha*mu*cw1[o]
            dt72 = mp.tile([6 * C1, gpc], F32, tag="dt72")
            nc.vector.tensor_scalar(
                out=dt72[:], in0=mual[0:6 * C1, gpc:2 * gpc],
                scalar1=cw1po_sb[0:6 * C1], scalar2=None, op0=OP.mult)
            nc.vector.tensor_scalar(
                out=dt72[:], in0=dt72[:], scalar1=-1.0,
                scalar2=b1ppo_sb[0:6 * C1], op0=OP.mult, op1=OP.add)

            # per-graph PE conv pipeline; y2b [q, (g, cc3, w, c)] fp16
            y2b = mp.tile([128, gpc * 3 * 6 * C2], F16, tag="y2b")
            with (
                tc.tile_pool(name="psg", bufs=2, space="PSUM") as psg,
                tc.tile_pool(name="psy", bufs=2, space="PSUM") as psy,
                tc.tile_pool(name="sg", bufs=3) as sg,
            ):
                for g4 in range(gpc // 4):
                    yps = psy.tile([128, 4 * 3 * 24], F16, tag="yps")
                    for gi in range(4):
                        g = 4 * g4 + gi
                        tp = psg.tile([96, 384], F16, tag="tp")
                        for c3 in range(3):
                            base = (NF * g + 6 * c3) * M
                            nc.tensor.matmul(
                                out=tp[:, c3 * 128:(c3 + 1) * 128],
                                lhsT=hr5[:, base:base + 6 * M],
                                rhs=id16_sb[:], is_transpose=True,
                                start=True, stop=True, skip_group_check=True)
                        hT = sg.tile([96, 384], F16, tag="hT")
                        nc.vector.tensor_copy(out=hT[:], in_=tp[:])
                        ups = psg.tile([6 * C1, 384], F32, tag="ups")
                        nc.tensor.matmul(out=ups[:], lhsT=bd1_sb[:],
                                         rhs=hT[:], start=True, stop=True)
                        y1r = sg.tile([6 * C1, 384], F16, tag="y1r")
                        nc.scalar.activation(
                            out=y1r[:], in_=ups[:], func=AF.Relu,
                            bias=dt72[:, g:g + 1],
                            scale=mual[0:6 * C1, g:g + 1])
                        y2p = psg.tile([6 * C2, 384], F32, tag="y2p")
                        nc.tensor.matmul(out=y2p[:], lhsT=bd2_sb[:],
                                         rhs=y1r[:], start=True, stop=True)
                        y2r = sg.tile([6 * C2, 384], F16, tag="y2r")
                        nc.vector.tensor_scalar(
                            out=y2r[:], in0=y2p[:],
                            scalar1=b2po_sb[0:6 * C2], scalar2=0.0,
                            op0=OP.add, op1=OP.max)
                        for c3 in range(3):
                            nc.tensor.matmul(
                                out=yps[:, 72 * gi + 24 * c3:
                                        72 * gi + 24 * (c3 + 1)],
                                lhsT=y2r[:, c3 * 128:(c3 + 1) * 128],
                                rhs=id16_sb[0:6 * C2, 0:6 * C2],
                                is_transpose=True, start=True, stop=True,
                                skip_group_check=True)
                    nc.vector.tensor_copy(
                        out=y2b[:, 288 * g4:288 * (g4 + 1)], in_=yps[:])

            # FC1 on PE (fp16, f32 PSUM accum over 72 chunks)
            ps_cm = tc.tile_pool(name="psfc", bufs=1, space="PSUM")
            ps = ps_cm.__enter__()
            y2bv = y2b[:].rearrange("q (g c3 w c) -> q g c3 w c",
                                    g=gpc, c3=3, w=6)
            psz = ps.tile([gpc, H1], F32, tag="psz")
            nk = C2 * NF
            ki = 0
            for c3 in range(3):
                for w in range(6):
                    for c in range(C2):
                        kk = c * NF + 6 * c3 + w
                        nc.tensor.matmul(
                            out=psz[:], lhsT=y2bv[:, :, c3, w, c],
                            rhs=fw1_sb[:, kk * H1:(kk + 1) * H1],
                            start=(ki == 0), stop=(ki == nk - 1))
                        ki += 1
            fb1p_t = ps.tile([gpc, H1], F32, tag="psB2")
            nc.tensor.matmul(out=fb1p_t[:], lhsT=onesr_sb[:, 0:gpc],
                             rhs=fb1_sb[:], start=True, stop=True)
            fb1b = mp.tile([gpc, H1], F32, tag="fb1b")
            nc.scalar.copy(out=fb1b[:], in_=fb1p_t[:])
            z1 = mp.tile([gpc, H1], F32, tag="z1")
            nc.vector.tensor_add(out=z1[:], in0=psz[:], in1=fb1b[:])
            nc.vector.tensor_scalar(
                out=z1[:], in0=z1[:], scalar1=0.0, scalar2=None, op0=OP.max)

            # FC2
            z1t = mp.tile([128, 2 * gpc], F32, tag="z1t")
            for k in range(2):
                pst2 = ps.tile([128, gpc], F32, tag="psB2")
                nc.tensor.transpose(
                    out=pst2[:], in_=z1[:, k * 128:(k + 1) * 128],
                    identity=ident_sb[0:gpc, 0:gpc])
                nc.vector.tensor_copy(
                    out=z1t[:, k * gpc:(k + 1) * gpc], in_=pst2[:])
            psz2 = ps.tile([gpc, H2], F32, tag="psz2")
            for k in range(2):
                nc.tensor.matmul(
                    out=psz2[:], lhsT=z1t[:, k * gpc:(k + 1) * gpc],
                    rhs=fw2_sb[:, k * H2:(k + 1) * H2],
                    start=(k == 0), stop=(k == 1))
            fb2p_t = ps.tile([gpc, H2], F32, tag="psB3")
            nc.tensor.matmul(out=fb2p_t[:], lhsT=onesr_sb[:, 0:gpc],
                             rhs=fb2_sb[:], start=True, stop=True)
            fb2b = mp.tile([gpc, H2], F32, tag="fb2b")
            nc.scalar.copy(out=fb2b[:], in_=fb2p_t[:])
            z2 = mp.tile([gpc, H2], F32, tag="z2")
            nc.vector.tensor_add(out=z2[:], in0=psz2[:], in1=fb2b[:])
            nc.vector.tensor_scalar(
                out=z2[:], in0=z2[:], scalar1=0.0, scalar2=None, op0=OP.max)

            # FC3
            psz2t = ps.tile([H2, gpc], F32, tag="psB2")
            nc.tensor.transpose(out=psz2t[:], in_=z2[:],
                                identity=ident_sb[0:gpc, 0:gpc])
            z2t = mp.tile([H2, gpc], F32, tag="z2t")
            nc.vector.tensor_copy(out=z2t[:], in_=psz2t[:])
            psz3 = ps.tile([gpc, 1], F32, tag="psB2")
            nc.tensor.matmul(out=psz3[:], lhsT=z2t[:], rhs=fw3_sb[:],
                             start=True, stop=True)
            zout = mp.tile([gpc, 1], F32, tag="zout")
            nc.vector.tensor_scalar(
                out=zout[:], in0=psz3[:], scalar1=fb3_sb[0:gpc, 0:1],
                scalar2=None, op0=OP.add)
            nc.sync.dma_start(out=out_p[:], in_=zout[:])
            ps_cm.__exit__(None, None, None)
    _split_excess_waits(nc)
    return nc


def _prep_edges(x, edge_index, edge_weight, gpc):
    """Sort edges by destination node, pad each node's list to K slots,
    lay out per-core [128, gpc*NF*K] fp16 arrays of source values and
    edge weights (node j of graph g at partition j%128, col NF*g+j//128)."""
    E = edge_index.shape[1]
    dst = edge_index[1].astype(np.int64)
    src = edge_index[0].astype(np.int64)
    counts = np.bincount(dst, minlength=B * N)
    K = 8
    while K < counts.max():
        K *= 2
    order = np.argsort(dst, kind="stable")
    ds = dst[order]
    starts = np.concatenate([[0], np.cumsum(counts)[:-1]])
    within = np.arange(E, dtype=np.int64) - np.repeat(starts, counts)
    xs = np.asarray(x, np.float32).ravel()[src[order]]
    ws = np.asarray(edge_weight, np.float32)[order]
    vx = np.zeros((B * N, K), np.float16)
    vw = np.zeros((B * N, K), np.float16)
    vx[ds, within] = xs.astype(np.float16)
    vw[ds, within] = ws.astype(np.float16)

    def lay(a):                                  # [B*N, K] -> per-core list
        ap = np.zeros((B, NPAD, K), np.float16)
        ap[:, :N] = a.reshape(B, N, K)
        ap = ap.reshape(B, NF, 128, K)
        outs = []
        for c in range(NCORES):
            s = ap[c * gpc:(c + 1) * gpc]        # [gpc, NF, 128, K]
            outs.append(np.ascontiguousarray(
                s.transpose(2, 0, 1, 3).reshape(128, gpc * NF * K)))
        return outs

    return lay(vx), lay(vw), K


def _layout_nodes(a, gpc):
    """[gpc, <=NPAD] -> [128, gpc*NF], node j at (j % 128, NF*g + j//128)."""
    a = np.asarray(a, np.float32)
    out = np.zeros((gpc, NF, 128), np.float32)
    out.reshape(gpc, -1)[:, :a.shape[1]] = a
    return np.ascontiguousarray(out.transpose(2, 0, 1).reshape(128, gpc * NF))


def _run(inputs, gpc, ncores):
    x = np.asarray(inputs["x"], np.float32)
    vxs, vws, K = _prep_edges(
        x, np.asarray(inputs["edge_index"]), inputs["edge_weight"], gpc)

    gf = lambda k: np.asarray(inputs[k], np.float32)
    w_root, w_rel, b_rel = gf("w_root"), gf("w_rel"), gf("b_rel")
    ln_g, ln_b = gf("ln_g"), gf("ln_b")
    gc1_w, gc1_b = gf("gc1_w"), gf("gc1_b")
    bn1_g, bn1_b = gf("bn1_g"), gf("bn1_b")
    gc2_w, gc2_b = gf("gc2_w"), gf("gc2_b")
    bn2_g, bn2_b = gf("bn2_g"), gf("bn2_b")
    fc_w1, fc_b1 = gf("fc_w1"), gf("fc_b1")
    fbn1_g, fbn1_b = gf("fbn1_g"), gf("fbn1_b")
    fc_w2, fc_b2 = gf("fc_w2"), gf("fc_b2")
    fbn2_g, fbn2_b = gf("fbn2_g"), gf("fbn2_b")
    fc1_w, fc1_b = gf("fc1_w"), gf("fc1_b")

    # LayerNorm gamma/beta must be channelwise for the conv1 fold
    assert np.all(ln_g == ln_g[0:1]) and np.all(ln_b == ln_b[0:1]), \
        "kernel requires channelwise LayerNorm affine"
    gam, bet = ln_g[0], ln_b[0]                          # [M]
    w1p = gc1_w * gam[None, :]                           # [C1, M]
    b1p = gc1_b + gc1_w @ bet                            # [C1]
    cw1 = w1p.sum(axis=1)                                # [C1]
    # pad-node LN-stat corrections: pad z = brel (agg=0, x=0)
    relu_b = np.maximum(b_rel, 0.0)
    pad_s = float((NPAD - N) * relu_b.sum())
    pad_q = float((NPAD - N) * (relu_b ** 2).sum())

    # fold eval-BN (rm=0, rv=1) into adjacent linear layers
    s1, t1 = BN_SCALE * bn1_g, bn1_b
    w2f = gc2_w * s1[None, :]
    b2f = gc2_b + gc2_w @ t1
    s2, t2 = BN_SCALE * bn2_g, bn2_b
    fw1p = np.zeros((C2, NPAD, H1), np.float32)
    fw1r = fc_w1.reshape(C2, N, H1)
    fw1p[:, :N] = fw1r * s2[:, None, None]
    fb1f = fc_b1 + np.einsum("c,cnh->h", t2, fw1r)
    sf1, tf1 = BN_SCALE * fbn1_g, fbn1_b
    fw1p *= sf1[None, None, :]
    fb1f = fb1f * sf1 + tf1
    sf2, tf2 = BN_SCALE * fbn2_g, fbn2_b
    fw2f = fc_w2 * sf2[None, :]
    fb2f = fc_b2 * sf2 + tf2

    fw1c = np.ascontiguousarray(
        fw1p.reshape(C2, NF, 128, H1).transpose(2, 0, 1, 3)
        .reshape(128, C2 * NF * H1)).astype(np.float16)
    fw2c = np.ascontiguousarray(
        fw2f.reshape(2, 128, H2).transpose(1, 0, 2).reshape(128, 2 * H2))

    # blockdiag conv weights: bd1[6m+w, 12w+o] = w1p[o, m]
    bd1a = np.zeros((6, M, 6, C1), np.float32)
    bd2a = np.zeros((6, C1, 6, C2), np.float32)
    for w in range(6):
        bd1a[w, :, w, :] = w1p.T
        bd2a[w, :, w, :] = w2f.T
    bd1c = bd1a.reshape(96, 6 * C1).astype(np.float16)
    bd2c = bd2a.reshape(6 * C1, 6 * C2).astype(np.float16)
    po = lambda v, r: np.concatenate(
        [np.tile(np.asarray(v, np.float32), 6),
         np.zeros(128 - 6 * len(v), np.float32)]).reshape(128, 1)

    nc = _build_program(gpc, K, pad_s, pad_q)

    common = {
        "wrel": np.ascontiguousarray(np.broadcast_to(
            w_rel.reshape(1, M), (128, M))),
        "wroot": np.ascontiguousarray(np.broadcast_to(
            w_root.reshape(1, M), (128, M))),
        "brel": np.ascontiguousarray(np.broadcast_to(
            b_rel.reshape(1, M), (128, M))),
        "bd1": bd1c, "bd2": bd2c,
        "cw1po": po(cw1, 128), "b1ppo": po(b1p, 128), "b2po": po(b2f, 128),
        "fw1": fw1c, "fb1": fb1f.reshape(1, H1),
        "fw2": fw2c, "fb2": fb2f.reshape(1, H2),
        "fw3": fc1_w.reshape(H2, 1),
        "ident": np.eye(128, dtype=np.float32),
        "ident16": np.eye(128, dtype=np.float16),
        "ones": np.ones((128, 1), np.float32),
        "ones_row": np.ones((1, 128), np.float32),
        "fb3": np.full((128, 1), float(np.ravel(fc1_b)[0]), np.float32),
    }
    in_maps = []
    for c in range(ncores):
        m = dict(common)
        m["vx"] = vxs[c]
        m["vw"] = vws[c]
        xl = np.zeros((gpc, NPAD), np.float32)
        xl[:, :N] = x.reshape(B, N)[c * gpc:(c + 1) * gpc]
        m["x128"] = _layout_nodes(xl, gpc).astype(np.float16)
        in_maps.append(m)

    res = run_bass_kernel_spmd(nc, in_maps, list(range(ncores)),
                               trace=TRACE)
    LAST["results"] = res
    out = np.concatenate([res.results[c]["out"] for c in range(ncores)],
                         axis=0)
    return out.astype(np.float32)


def kernel(**inputs):
    return _run(inputs, B // NCORES, NCORES)


# revision 12
# speedup vs baseline: 1.1044x; 1.1044x over previous
"""CSGNet (gnn_message_passing) Trainium2 kernel, v3.

Sharding (per hint): data-parallel over graphs, 32 graphs per core.

Phase 1 (scatter-free aggregation): the host sorts edges by destination
node and pads each node's edge list to K slots (K = next pow2 >= max
degree), so GraphConv aggregation becomes a dense per-node row sum. The
device streams fp16 source-value and weight arrays, multiplies (DVE 2x
mode), and sums the K slots with a pairwise add tree (tensor_tensor gets
the 2x fp16 mode; tensor_reduce does not).

Phase 2: GraphConv combine in fp16 on DVE; relu + LayerNorm stats fused
on the scalar engine (accum_out); LayerNorm folded into conv1 (channel-
wise ln_g/ln_b checked on host). conv1/conv2 run on the PE as block-
diagonal matmuls over PE-transposed per-graph tiles (6-node packing);
the LN affine + relu epilogue is fused into scalar-engine activations
reading PSUM. FC stack on PE with fp16 fc_w1. Eval-BatchNorms folded on
host.
"""

import numpy as np

import concourse.bass as bass
import concourse.mybir as mybir
from concourse.tile import TileContext
from concourse.vector_clock import ScopedClock
from concourse.bass_utils import run_bass_kernel_spmd

F32 = mybir.dt.float32
F16 = mybir.dt.float16
OP = mybir.AluOpType
AX = mybir.AxisListType
AF = mybir.ActivationFunctionType

B, N, M = 256, 2207, 16
C1, C2 = 12, 4
H1, H2 = 256, 64
EPS = 1e-5
BN_SCALE = 1.0 / np.sqrt(1.0 + 1e-5)
NCORES = 8

NF = 18                      # node j of graph g at (j % 128, NF*g + j // 128)
NPAD = NF * 128              # 2304 padded nodes per graph
CW = 48                      # node-columns per phase-1 chunk (576 / 12)
TRACE = False                # capture NTFF profile (test harness only)
LAST = {}                    # test harness: last run artifacts


# ---------------------------------------------------------------------------
# workaround: this walrus build rejects >2 sem waits on one TPB_CTRL
# instruction; spread the TileContext tail-drain waits over 1-wait nops.
def _patched_drain_and_barrier(self, tick_clock, wait_clock):
    probe = self.nc.sync.nop(nofuse=True)
    wait_clock.add_sem_waits(probe.ins, ScopedClock({None: tick_clock.global_clock}))
    si = probe.ins.sync_info
    waits = list(si.on_wait) if si is not None and si.on_wait else []
    if len(waits) > 1:
        si.on_wait.clear()
        si.on_wait.append(waits[0])
        for w in waits[1:]:
            n2 = self.nc.sync.nop(nofuse=True)
            n2.ins.sync_info = mybir.SyncInfo(on_wait=[w], on_update=[])
    self.nc.sync.drain()
    self.nc.all_engine_barrier()
    popped = self.nc._tile_sem_poison_stack.pop()
    assert popped is self._sem_poison
    self.nc.clear_and_free_semaphores(list(self.sems.allocated().values()))
    self.nc.all_engine_barrier()


TileContext._drain_and_barrier = _patched_drain_and_barrier


def _split_excess_waits(nc, limit=1):
    """Walrus caps sem waits per instruction; move extras to same-engine
    nops placed immediately before the offending instruction."""
    n = 0
    for fn in nc.m.functions:
        for bb in fn.blocks:
            insts = bb.instructions
            out = []
            changed = False
            for inst in insts:
                si = inst.sync_info
                if si is not None and si.on_wait and len(si.on_wait) > limit:
                    waits = list(si.on_wait)
                    extra, keep = waits[:-limit], waits[-limit:]
                    for i in range(0, len(extra), limit):
                        n += 1
                        out.append(mybir.InstNoOp(
                            name=f"ZZwait-{n}", engine=inst.engine,
                            sync_info=mybir.SyncInfo(
                                on_wait=extra[i:i + limit], on_update=[])))
                    inst.sync_info = mybir.SyncInfo(
                        on_wait=keep, on_update=list(si.on_update or []))
                    changed = True
                out.append(inst)
            if changed:
                bb.instructions = out
# ---------------------------------------------------------------------------


def _build_program(gpc, K, pad_s, pad_q):
    """SPMD Tile program. gpc graphs/core, K slots/node (power of 2)."""
    GF = gpc * NF               # 576 node-columns per core
    nch = GF // CW
    assert GF % CW == 0 and (K & (K - 1)) == 0

    nc = bass.Bass()
    dp = lambda n, s, d=F32: nc.declare_dram_parameter(n, s, d, isOutput=False)

    vx = dp("vx", [128, GF * K], F16)
    vw = dp("vw", [128, GF * K], F16)
    x128 = dp("x128", [128, GF])
    wrel = dp("wrel", [128, M])
    wroot = dp("wroot", [128, M])
    brel = dp("brel", [128, M])
    bd1 = dp("bd1", [96, 6 * C1], F16)    # blockdiag W1' (LN-gamma folded)
    bd2 = dp("bd2", [6 * C1, 6 * C2], F16)  # blockdiag BN1-folded gc2_w
    cw1po = dp("cw1po", [128, 1])         # row sums of W1' at part 12w+o
    b1ppo = dp("b1ppo", [128, 1])         # b1' at part 12w+o
    b2po = dp("b2po", [128, 1])           # folded conv2 bias at part 4w+c
    fw1 = dp("fw1", [128, (C2 * NF) * H1], F16)
    fb1 = dp("fb1", [1, H1])
    fw2 = dp("fw2", [128, 2 * H2])
    fb2 = dp("fb2", [1, H2])
    fw3 = dp("fw3", [64, 1])
    ident = dp("ident", [128, 128])
    ident16 = dp("ident16", [128, 128], F16)
    ones = dp("ones", [128, 1])
    ones_row = dp("ones_row", [1, 128])
    fb3 = dp("fb3", [128, 1])
    out_p = nc.declare_dram_parameter("out", [gpc, 1], F32, isOutput=True)

    with TileContext(nc) as tc:
        with (
            tc.tile_pool(name="const", bufs=1) as cpool,
            tc.tile_pool(name="main", bufs=1) as mp,
        ):
            # big weight prefetch first so it overlaps phase 1
            fw1_sb = cpool.tile([128, (C2 * NF) * H1], F16)
            nc.sync.dma_start(out=fw1_sb[:], in_=fw1[:])

            def ld(t, shape, dt=F32):
                s = cpool.tile(list(shape), dt, tag=t.name)
                nc.sync.dma_start(out=s[:], in_=t[:])
                return s

            ident_sb = ld(ident, [128, 128])
            id16_sb = ld(ident16, [128, 128], F16)
            ones_sb = ld(ones, [128, 1])
            onesr_sb = ld(ones_row, [1, 128])
            x_sb = ld(x128, [128, GF])
            wrel_sb = ld(wrel, [128, M])
            wroot_sb = ld(wroot, [128, M])
            brel_sb = ld(brel, [128, M])
            bd1_sb = ld(bd1, [96, 6 * C1], F16)
            bd2_sb = ld(bd2, [6 * C1, 6 * C2], F16)
            cw1po_sb = ld(cw1po, [128, 1])
            b1ppo_sb = ld(b1ppo, [128, 1])
            b2po_sb = ld(b2po, [128, 1])
            fb1_sb = ld(fb1, [1, H1])
            fw2_sb = ld(fw2, [128, 2 * H2])
            fb2_sb = ld(fb2, [1, H2])
            fw3_sb = ld(fw3, [64, 1])
            fb3_sb = ld(fb3, [128, 1])

            agg = mp.tile([128, GF], F32, tag="agg")

            # -------- Phase 1: fp16 multiply + pairwise-tree slot sums -----
            with (
                tc.tile_pool(name="edges", bufs=3) as epool,
                tc.tile_pool(name="prod", bufs=2) as ppool,
            ):
                for ch in range(nch):
                    c0 = ch * CW
                    vx_t = epool.tile([128, CW * K], F16, tag="vx")
                    vw_t = epool.tile([128, CW * K], F16, tag="vw")
                    nc.sync.dma_start(
                        out=vx_t[:], in_=vx[:, c0 * K:(c0 + CW) * K])
                    nc.sync.dma_start(
                        out=vw_t[:], in_=vw[:, c0 * K:(c0 + CW) * K])
                    prod = ppool.tile([128, CW * K], F16, tag="prod")
                    nc.vector.tensor_mul(out=prod[:], in0=vx_t[:], in1=vw_t[:])
                    p3 = prod[:].rearrange("q (c k) -> q c k", k=K)
                    with nc.allow_low_precision(reason="fp16 slot-sum tree"):
                        k = K
                        while k > 2:
                            h = k // 2
                            nc.vector.tensor_tensor(
                                out=p3[:, :, 0:h], in0=p3[:, :, 0:h],
                                in1=p3[:, :, h:k], op=OP.add)
                            k = h
                    nc.vector.tensor_tensor(
                        out=agg[:, c0:c0 + CW], in0=p3[:, :, 0],
                        in1=p3[:, :, 1], op=OP.add)

            # -------- Phase 2: combine, LN stats, PE convs, FC stack -------
            agg16 = mp.tile([128, GF], F16, tag="agg16")
            nc.vector.tensor_copy(out=agg16[:], in_=agg[:])

            # z[m, n] = agg[n]*wrel[m] + x[n]*wroot[m] + brel[m]  (pre-relu)
            h5 = mp.tile([128, M * GF], F16, tag="h5")      # [q, m, gf]
            h5v = h5[:].rearrange("q (gf m) -> q gf m", m=M)
            tmp = mp.tile([128, GF], F16, tag="tmp")
            for m in range(M):
                nc.vector.tensor_scalar(
                    out=tmp[:], in0=x_sb[:],
                    scalar1=wroot_sb[:, m:m + 1], scalar2=brel_sb[:, m:m + 1],
                    op0=OP.mult, op1=OP.add)
                nc.vector.scalar_tensor_tensor(
                    out=h5v[:, :, m], in0=agg16[:],
                    scalar=wrel_sb[:, m:m + 1], in1=tmp[:],
                    op0=OP.mult, op1=OP.add)

            # relu + per-graph LN stats on the scalar engine (accum_out)
            hr5 = mp.tile([128, M * GF], F16, tag="hr5")
            hr5v = hr5[:].rearrange("q (gf m) -> q gf m", m=M)
            ssum = mp.tile([128, 2 * gpc], F32, tag="ssum")
            with tc.tile_pool(name="sq", bufs=2) as sqpool:
                for g in range(gpc):
                    sl = slice(g * NF, (g + 1) * NF)
                    nc.scalar.activation(
                        out=hr5v[:, sl], in_=h5v[:, sl], func=AF.Relu,
                        accum_out=ssum[:, g:g + 1])
                    sq = sqpool.tile([128, M * NF], F16, tag="sq")
                    nc.scalar.activation(
                        out=sq[:].rearrange("q (f m) -> q f m", m=M),
                        in_=hr5v[:, sl], func=AF.Square,
                        accum_out=ssum[:, gpc + g:gpc + g + 1])

            # mual[128, 64] = alpha | alpha*mu (broadcast over partitions)
            ps1_cm = tc.tile_pool(name="ps1", bufs=1, space="PSUM")
            ps1 = ps1_cm.__enter__()
            pstat = ps1.tile([1, 2 * gpc], F32, tag="psA")
            nc.tensor.matmul(out=pstat[:], lhsT=ones_sb[:], rhs=ssum[:],
                             start=True, stop=True)
            inv = 1.0 / (N * M)
            mu1 = mp.tile([1, gpc], F32, tag="mu1")
            nc.vector.tensor_scalar(
                out=mu1[:], in0=pstat[:, 0:gpc],
                scalar1=-pad_s, scalar2=inv, op0=OP.add, op1=OP.mult)
            e2 = mp.tile([1, gpc], F32, tag="e2")
            nc.vector.tensor_scalar(
                out=e2[:], in0=pstat[:, gpc:2 * gpc],
                scalar1=-pad_q, scalar2=inv, op0=OP.add, op1=OP.mult)
            musq = mp.tile([1, gpc], F32, tag="musq")
            nc.vector.tensor_mul(out=musq[:], in0=mu1[:], in1=mu1[:])
            nc.vector.tensor_sub(out=e2[:], in0=e2[:], in1=musq[:])
            nc.vector.tensor_scalar(
                out=e2[:], in0=e2[:], scalar1=EPS, scalar2=None, op0=OP.add)
            sd1 = mp.tile([1, gpc], F32, tag="sd1")
            nc.scalar.sqrt(out=sd1[:], in_=e2[:])
            mual1 = mp.tile([1, 2 * gpc], F32, tag="mual1")
            nc.vector.reciprocal(out=mual1[:, 0:gpc], in_=sd1[:])
            nc.vector.tensor_mul(out=mual1[:, gpc:2 * gpc],
                                 in0=mual1[:, 0:gpc], in1=mu1[:])
            mualp = ps1.tile([128, 2 * gpc], F32, tag="psA2")
            nc.tensor.matmul(out=mualp[:], lhsT=onesr_sb[:], rhs=mual1[:],
                             start=True, stop=True)
            mual = mp.tile([128, 2 * gpc], F32, tag="mual")
            nc.vector.tensor_copy(out=mual[:], in_=mualp[:])
            ps1_cm.__exit__(None, None, None)

            # conv1 epilogue bias D[12w+o, g] = b1p[o] - alpha*mu*cw1[o]
            dt72 = mp.tile([6 * C1, gpc], F32, tag="dt72")
            nc.vector.tensor_scalar(
                out=dt72[:], in0=mual[0:6 * C1, gpc:2 * gpc],
                scalar1=cw1po_sb[0:6 * C1], scalar2=None, op0=OP.mult)
            nc.vector.tensor_scalar(
                out=dt72[:], in0=dt72[:], scalar1=-1.0,
                scalar2=b1ppo_sb[0:6 * C1], op0=OP.mult, op1=OP.add)

            # per-graph PE conv pipeline; y2b [q, (g, cc3, w, c)] fp16
            y2b = mp.tile([128, gpc * 3 * 6 * C2], F16, tag="y2b")
            with (
                tc.tile_pool(name="psg", bufs=2, space="PSUM") as psg,
                tc.tile_pool(name="psy", bufs=2, space="PSUM") as psy,
                tc.tile_pool(name="sg", bufs=3) as sg,
            ):
                for g4 in range(gpc // 4):
                    yps = psy.tile([128, 4 * 3 * 24], F16, tag="yps")
                    for gi in range(4):
                        g = 4 * g4 + gi
                        tp = psg.tile([96, 384], F16, tag="tp")
                        for c3 in range(3):
                            base = (NF * g + 6 * c3) * M
                            nc.tensor.matmul(
                                out=tp[:, c3 * 128:(c3 + 1) * 128],
                                lhsT=hr5[:, base:base + 6 * M],
                                rhs=id16_sb[:], is_transpose=True,
                                start=True, stop=True, skip_group_check=True)
                        hT = sg.tile([96, 384], F16, tag="hT")
                        nc.vector.tensor_copy(out=hT[:], in_=tp[:])
                        ups = psg.tile([6 * C1, 384], F32, tag="ups")
                        nc.tensor.matmul(out=ups[:], lhsT=bd1_sb[:],
                                         rhs=hT[:], start=True, stop=True)
                        y1r = sg.tile([6 * C1, 384], F16, tag="y1r")
                        nc.scalar.activation(
                            out=y1r[:], in_=ups[:], func=AF.Relu,
                            bias=dt72[:, g:g + 1],
                            scale=mual[0:6 * C1, g:g + 1])
                        y2p = psg.tile([6 * C2, 384], F32, tag="y2p")
                        nc.tensor.matmul(out=y2p[:], lhsT=bd2_sb[:],
                                         rhs=y1r[:], start=True, stop=True)
                        y2r = sg.tile([6 * C2, 384], F16, tag="y2r")
                        nc.vector.tensor_scalar(
                            out=y2r[:], in0=y2p[:],
                            scalar1=b2po_sb[0:6 * C2], scalar2=0.0,
                            op0=OP.add, op1=OP.max)
                        for c3 in range(3):
                            nc.tensor.matmul(
                                out=yps[:, 72 * gi + 24 * c3:
                                        72 * gi + 24 * (c3 + 1)],
                                lhsT=y2r[:, c3 * 128:(c3 + 1) * 128],
                                rhs=id16_sb[0:6 * C2, 0:6 * C2],
                                is_transpose=True, start=True, stop=True,
                                skip_group_check=True)
                    nc.vector.tensor_copy(
                        out=y2b[:, 288 * g4:288 * (g4 + 1)], in_=yps[:])

            # FC1 on PE (fp16, f32 PSUM accum over 72 chunks)
            ps_cm = tc.tile_pool(name="psfc", bufs=1, space="PSUM")
            ps = ps_cm.__enter__()
            y2bv = y2b[:].rearrange("q (g c3 w c) -> q g c3 w c",
                                    g=gpc, c3=3, w=6)
            psz = ps.tile([gpc, H1], F32, tag="psz")
            nk = C2 * NF
            ki = 0
            for c3 in range(3):
                for w in range(6):
                    for c in range(C2):
                        kk = c * NF + 6 * c3 + w
                        nc.tensor.matmul(
                            out=psz[:], lhsT=y2bv[:, :, c3, w, c],
                            rhs=fw1_sb[:, kk * H1:(kk + 1) * H1],
                            start=(ki == 0), stop=(ki == nk - 1))
                        ki += 1
            fb1p_t = ps.tile([gpc, H1], F32, tag="psB2")
            nc.tensor.matmul(out=fb1p_t[:], lhsT=onesr_sb[:, 0:gpc],
                             rhs=fb1_sb[:], start=True, stop=True)
            fb1b = mp.tile([gpc, H1], F32, tag="fb1b")
            nc.scalar.copy(out=fb1b[:], in_=fb1p_t[:])
            z1 = mp.tile([gpc, H1], F32, tag="z1")
            nc.vector.tensor_add(out=z1[:], in0=psz[:], in1=fb1b[:])
            nc.vector.tensor_scalar(
                out=z1[:], in0=z1[:], scalar1=0.0, scalar2=None, op0=OP.max)

            # FC2
            z1t = mp.tile([128, 2 * gpc], F32, tag="z1t")
            for k in range(2):
                pst2 = ps.tile([128, gpc], F32, tag="psB2")
                nc.tensor.transpose(
                    out=pst2[:], in_=z1[:, k * 128:(k + 1) * 128],
                    identity=ident_sb[0:gpc, 0:gpc])
                nc.vector.tensor_copy(
                    out=z1t[:, k * gpc:(k + 1) * gpc], in_=pst2[:])
            psz2 = ps.tile([gpc, H2], F32, tag="psz2")
            for k in range(2):
                nc.tensor.matmul(
                    out=psz2[:], lhsT=z1t[:, k * gpc:(k + 1) * gpc],
                    rhs=fw2_sb[:, k * H2:(k + 1) * H2],
                    start=(k == 0), stop=(k == 1))
            fb2p_t = ps.tile([gpc, H2], F32, tag="psB3")
            nc.tensor.matmul(out=fb2p_t[:], lhsT=onesr_sb[:, 0:gpc],
                             rhs=fb2_sb[:], start=True, stop=True)
            fb2b = mp.tile([gpc, H2], F32, tag="fb2b")
            nc.scalar.copy(out=fb2b[:], in_=fb2p_t[:])
            z2 = mp.tile([gpc, H2], F32, tag="z2")
            nc.vector.tensor_add(out=z2[:], in0=psz2[:], in1=fb2b[:])
            nc.vector.tensor_scalar(
                out=z2[:], in0=z2[:], scalar1=0.0, scalar2=None, op0=OP.max)

            # FC3
            psz2t = ps.tile([H2, gpc], F32, tag="psB2")
            nc.tensor.transpose(out=psz2t[:], in_=z2[:],
                                identity=ident_sb[0:gpc, 0:gpc])
            z2t = mp.tile([H2, gpc], F32, tag="z2t")
            nc.vector.tensor_copy(out=z2t[:], in_=psz2t[:])
            psz3 = ps.tile([gpc, 1], F32, tag="psB2")
            nc.tensor.matmul(out=psz3[:], lhsT=z2t[:], rhs=fw3_sb[:],
                             start=True, stop=True)
            zout = mp.tile([gpc, 1], F32, tag="zout")
            nc.vector.tensor_scalar(
                out=zout[:], in0=psz3[:], scalar1=fb3_sb[0:gpc, 0:1],
                scalar2=None, op0=OP.add)
            nc.sync.dma_start(out=out_p[:], in_=zout[:])
            ps_cm.__exit__(None, None, None)
    _split_excess_waits(nc)
    return nc


def _prep_edges(x, edge_index, edge_weight, gpc):
    """Sort edges by destination node, pad each node's list to K slots,
    lay out per-core [128, gpc*NF*K] fp16 arrays of source values and
    edge weights (node j of graph g at partition j%128, col NF*g+j//128)."""
    E = edge_index.shape[1]
    dst = edge_index[1].astype(np.int64)
    src = edge_index[0].astype(np.int64)
    counts = np.bincount(dst, minlength=B * N)
    K = 8
    while K < counts.max():
        K *= 2
    order = np.argsort(dst, kind="stable")
    ds = dst[order]
    starts = np.concatenate([[0], np.cumsum(counts)[:-1]])
    within = np.arange(E, dtype=np.int64) - np.repeat(starts, counts)
    xs = np.asarray(x, np.float32).ravel()[src[order]]
    ws = np.asarray(edge_weight, np.float32)[order]
    vx = np.zeros((B * N, K), np.float16)
    vw = np.zeros((B * N, K), np.float16)
    vx[ds, within] = xs.astype(np.float16)
    vw[ds, within] = ws.astype(np.float16)

    def lay(a):                                  # [B*N, K] -> per-core list
        ap = np.zeros((B, NPAD, K), np.float16)
        ap[:, :N] = a.reshape(B, N, K)
        ap = ap.reshape(B, NF, 128, K)
        outs = []
        for c in range(NCORES):
            s = ap[c * gpc:(c + 1) * gpc]        # [gpc, NF, 128, K]
            outs.append(np.ascontiguousarray(
                s.transpose(2, 0, 1, 3).reshape(128, gpc * NF * K)))
        return outs

    return lay(vx), lay(vw), K


def _layout_nodes(a, gpc):
    """[gpc, <=NPAD] -> [128, gpc*NF], node j at (j % 128, NF*g + j//128)."""
    a = np.asarray(a, np.float32)
    out = np.zeros((gpc, NF, 128), np.float32)
    out.reshape(gpc, -1)[:, :a.shape[1]] = a
    return np.ascontiguousarray(out.transpose(2, 0, 1).reshape(128, gpc * NF))


def _run(inputs, gpc, ncores):
    x = np.asarray(inputs["x"], np.float32)
    vxs, vws, K = _prep_edges(
        x, np.asarray(inputs["edge_index"]), inputs["edge_weight"], gpc)

    gf = lambda k: np.asarray(inputs[k], np.float32)
    w_root, w_rel, b_rel = gf("w_root"), gf("w_rel"), gf("b_rel")
    ln_g, ln_b = gf("ln_g"), gf("ln_b")
    gc1_w, gc1_b = gf("gc1_w"), gf("gc1_b")
    bn1_g, bn1_b = gf("bn1_g"), gf("bn1_b")
    gc2_w, gc2_b = gf("gc2_w"), gf("gc2_b")
    bn2_g, bn2_b = gf("bn2_g"), gf("bn2_b")
    fc_w1, fc_b1 = gf("fc_w1"), gf("fc_b1")
    fbn1_g, fbn1_b = gf("fbn1_g"), gf("fbn1_b")
    fc_w2, fc_b2 = gf("fc_w2"), gf("fc_b2")
    fbn2_g, fbn2_b = gf("fbn2_g"), gf("fbn2_b")
    fc1_w, fc1_b = gf("fc1_w"), gf("fc1_b")

    # LayerNorm gamma/beta must be channelwise for the conv1 fold
    assert np.all(ln_g == ln_g[0:1]) and np.all(ln_b == ln_b[0:1]), \
        "kernel requires channelwise LayerNorm affine"
    gam, bet = ln_g[0], ln_b[0]                          # [M]
    w1p = gc1_w * gam[None, :]                           # [C1, M]
    b1p = gc1_b + gc1_w @ bet                            # [C1]
    cw1 = w1p.sum(axis=1)                                # [C1]
    # pad-node LN-stat corrections: pad z = brel (agg=0, x=0)
    relu_b = np.maximum(b_rel, 0.0)
    pad_s = float((NPAD - N) * relu_b.sum())
    pad_q = float((NPAD - N) * (relu_b ** 2).sum())

    # fold eval-BN (rm=0, rv=1) into adjacent linear layers
    s1, t1 = BN_SCALE * bn1_g, bn1_b
    w2f = gc2_w * s1[None, :]
    b2f = gc2_b + gc2_w @ t1
    s2, t2 = BN_SCALE * bn2_g, bn2_b
    fw1p = np.zeros((C2, NPAD, H1), np.float32)
    fw1r = fc_w1.reshape(C2, N, H1)
    fw1p[:, :N] = fw1r * s2[:, None, None]
    fb1f = fc_b1 + np.einsum("c,cnh->h", t2, fw1r)
    sf1, tf1 = BN_SCALE * fbn1_g, fbn1_b
    fw1p *= sf1[None, None, :]
    fb1f = fb1f * sf1 + tf1
    sf2, tf2 = BN_SCALE * fbn2_g, fbn2_b
    fw2f = fc_w2 * sf2[None, :]
    fb2f = fc_b2 * sf2 + tf2

    fw1c = np.ascontiguousarray(
        fw1p.reshape(C2, NF, 128, H1).transpose(2, 0, 1, 3)
        .reshape(128, C2 * NF * H1)).astype(np.float16)
    fw2c = np.ascontiguousarray(
        fw2f.reshape(2, 128, H2).transpose(1, 0, 2).reshape(128, 2 * H2))

    # blockdiag conv weights: bd1[6m+w, 12w+o] = w1p[o, m]
    bd1a = np.zeros((6, M, 6, C1), np.float32)
    bd2a = np.zeros((6, C1, 6, C2), np.float32)
    for w in range(6):
        bd1a[w, :, w, :] = w1p.T
        bd2a[w, :, w, :] = w2f.T
    bd1c = bd1a.reshape(96, 6 * C1).astype(np.float16)
    bd2c = bd2a.reshape(6 * C1, 6 * C2).astype(np.float16)
    po = lambda v, r: np.concatenate(
        [np.tile(np.asarray(v, np.float32), 6),
         np.zeros(128 - 6 * len(v), np.float32)]).reshape(128, 1)

    nc = _build_program(gpc, K, pad_s, pad_q)

    common = {
        "wrel": np.ascontiguousarray(np.broadcast_to(
            w_rel.reshape(1, M), (128, M))),
        "wroot": np.ascontiguousarray(np.broadcast_to(
            w_root.reshape(1, M), (128, M))),
        "brel": np.ascontiguousarray(np.broadcast_to(
            b_rel.reshape(1, M), (128, M))),
        "bd1": bd1c, "bd2": bd2c,
        "cw1po": po(cw1, 128), "b1ppo": po(b1p, 128), "b2po": po(b2f, 128),
        "fw1": fw1c, "fb1": fb1f.reshape(1, H1),
        "fw2": fw2c, "fb2": fb2f.reshape(1, H2),
        "fw3": fc1_w.reshape(H2, 1),
        "ident": np.eye(128, dtype=np.float32),
        "ident16": np.eye(128, dtype=np.float16),
        "ones": np.ones((128, 1), np.float32),
        "ones_row": np.ones((1, 128), np.float32),
        "fb3": np.full((128, 1), float(np.ravel(fc1_b)[0]), np.float32),
    }
    in_maps = []
    for c in range(ncores):
        m = dict(common)
        m["vx"] = vxs[c]
        m["vw"] = vws[c]
        xl = np.zeros((gpc, NPAD), np.float32)
        xl[:, :N] = x.reshape(B, N)[c * gpc:(c + 1) * gpc]
        m["x128"] = _layout_nodes(xl, gpc)
        in_maps.append(m)

    res = run_bass_kernel_spmd(nc, in_maps, list(range(ncores)),
                               trace=TRACE)
    LAST["results"] = res
    out = np.concatenate([res.results[c]["out"] for c in range(ncores)],
                         axis=0)
    return out.astype(np.float32)


def kernel(**inputs):
    return _run(inputs, B // NCORES, NCORES)
